# revision 11
# baseline (speedup 1.0000x reference)
"""Trainium2 Bass kernel for nn_ExtractorMLP (GNN edge cosine-similarity).

Math:  out[e] = cos_sim(mlp(emb[col[e]]), mlp(emb[row[e]]))
where  mlp(x) = elu(x @ W1.T + b1) @ W2.T + b2   (b1 = b2 = 0 for this problem)

Strategy (edge-data-parallel, SWDGE-token-halving):
  * Phase 1 (per node, replicated on every core): compute the normalized MLP
    output table  t[v] = g[v] / max(||g[v]||, eps)  in SBUF, node-major
    (node v in partition v%128, features at free cols [(v//128)*128, +128)),
    normalised per row-QUARTER so phase-2 gathers can start early.
  * Phase 2 (edges, sharded 8 ways): edges are grouped by
    (row-quarter q, col-block b) with group sizes equalised across cores
    (sizes baked into the program, so it stays SPMD).
      - row side: dma_gather (SWDGE) of t[row] from the quarter's table
        slice with int16 quarter-local indices  -> f2 [128 feats, cols]
      - col side: NO gather.  The host ships one-hot matrices; a per-block
        matmul  psF1 = table_block^T @ onehot  expands t[col] on the tensor
        engine (~0.7ns/token vs ~9ns/token on SWDGE).
      - prod = psF1 * f2 (DVE, PSUM x SBUF), then the sliding-one-hot
        reduce matmul contracts features into per-edge dots.

ELU identity used on device:  elu(x) = max(exp(min(x, 0)) - 1, x)
"""

import math

import numpy as np
import ml_dtypes

BF16 = ml_dtypes.bfloat16

H = 128          # feature dim
P = 128          # partitions
CHUNK = 512      # edges per reduce-matmul / PSUM bank width
GT = 4096        # edges per dma_gather instruction
NCORES = 8
NSWQ = 1
ST_W = 512       # phase-1 supertile width (nodes)
QS = 12544       # row-quarter size (multiple of 128, < 32768 for int16)

_PROG_CACHE: dict = {}
LAST_RESULTS = None  # test harness can inspect exec_time_ns


def _quarter_bounds(n_pad):
    """Uneven row segments, each <= 32767 (int16) and 128-aligned. A small
    first segment lets phase-2 gathers start early in phase 1; later segment
    sizes are paced so the phase-1 producer stays ahead of the gathers."""
    sizes = [4096, 10240, 16384]
    qb = [0]
    for s in sizes:
        if qb[-1] + s >= n_pad:
            break
        qb.append(qb[-1] + s)
    while n_pad - qb[-1] > 32640:     # safety for larger n_pad
        qb.append(qb[-1] + 32640)
    qb.append(n_pad)
    return qb  # len nq+1


def _build_program(n_pad, layout, trace_label=""):
    """Build the (shared, SPMD) bass program.

    layout: tuple of (q, b, start, size) per group, in stream order, plus
            per-quarter padded substream bounds; identical across cores.
    """
    import concourse.bacc as bacc
    import concourse.mybir as mybir
    import concourse.tile as tile
    from contextlib import ExitStack

    f32 = mybir.dt.float32
    bf16 = mybir.dt.bfloat16
    i16 = mybir.dt.int16
    Alu = mybir.AluOpType
    Act = mybir.ActivationFunctionType

    groups, qstart, qend = layout       # groups: list of (q, b, start, size)
    qb = _quarter_bounds(n_pad)
    nq = len(qb) - 1
    s_pad = qend[-1]                    # padded stream length
    n_chunks = s_pad // CHUNK
    n_groups_out = math.ceil(n_chunks / P)
    n_blocks = n_pad // H

    nc = bacc.Bacc("TRN2", target_bir_lowering=False, debug=False,
                   num_devices=NCORES, num_swdge_queues=NSWQ)

    embT = nc.dram_tensor("embT", [P, n_pad], bf16, kind="ExternalInput")
    w1t_d = nc.dram_tensor("w1t", [H, H], bf16, kind="ExternalInput")
    w2t_d = nc.dram_tensor("w2t", [H, H], bf16, kind="ExternalInput")
    oh_d = nc.dram_tensor("oh", [P, s_pad], bf16, kind="ExternalInput")
    ridx_d = nc.dram_tensor("ridx", [P, s_pad // 16], i16, kind="ExternalInput")
    out_d = nc.dram_tensor("out", [n_groups_out, P, CHUNK], f32,
                           kind="ExternalOutput")

    # group segments cut at CHUNK boundaries: per chunk, list of (b, lo, hi)
    chunk_segs = [[] for _ in range(n_chunks)]
    for (q, b, start, size) in groups:
        lo = start
        while lo < start + size:
            c = lo // CHUNK
            hi = min(start + size, (c + 1) * CHUNK)
            chunk_segs[c].append((b, lo - c * CHUNK, hi - c * CHUNK))
            lo = hi

    with ExitStack() as ctx:
        tc = ctx.enter_context(tile.TileContext(nc))
        const = ctx.enter_context(tc.tile_pool(name="const", bufs=1))
        p1 = ctx.enter_context(tc.tile_pool(name="p1", bufs=3))
        p2 = ctx.enter_context(tc.tile_pool(name="p2", bufs=3))
        post = ctx.enter_context(tc.tile_pool(name="post", bufs=2))
        poh = ctx.enter_context(tc.tile_pool(name="poh", bufs=3))
        pprod = ctx.enter_context(tc.tile_pool(name="pprod", bufs=GT // CHUNK))
        ps1 = ctx.enter_context(tc.tile_pool(name="ps1", bufs=2, space="PSUM"))
        ps2 = ctx.enter_context(tc.tile_pool(name="ps2", bufs=2, space="PSUM"))
        psf = ctx.enter_context(tc.tile_pool(name="psf", bufs=2, space="PSUM"))
        pso = ctx.enter_context(tc.tile_pool(name="pso", bufs=2, space="PSUM"))

        # --- constants / persistent tiles ---
        table = const.tile([P, n_pad], bf16, tag="table")
        w1t = const.tile([H, H], bf16, tag="w1t")
        w2t = const.tile([H, H], bf16, tag="w2t")
        # sliding one-hot: onehot[:, 127-p : 255-p] has ones in column p only;
        # used as lhsT so chunk p's dot-row lands in PSUM partition p.
        onehot = const.tile([P, 2 * P - 1], bf16, tag="onehot")
        ss_all = const.tile([P, n_blocks], f32, tag="ss_all")
        r_all = const.tile([P, n_blocks], f32, tag="r_all")
        s_all = const.tile([P, n_blocks], f32, tag="s_all")
        m_all = const.tile([P, n_blocks], f32, tag="m_all")
        ridx = const.tile([P, s_pad // 16], i16, tag="ridx")
        nc.sync.dma_start(out=w1t[:], in_=w1t_d[:])
        nc.sync.dma_start(out=w2t[:], in_=w2t_d[:])
        nc.sync.dma_start(out=ridx[:], in_=ridx_d[:])
        nc.vector.memset(onehot[:], 0.0)
        nc.vector.memset(onehot[:, P - 1:P], 1.0)

        # --- phase 1: MLP table (node-major), per-quarter normalization ---
        def phase1_quarter(qi):
            n0 = qb[qi]
            qhi = qb[qi + 1]
            st_base = n0 // H
            while n0 < qhi:
                w = min(ST_W, qhi - n0)
                nb = w // H
                xt = p1.tile([P, ST_W], bf16, tag="xt", name="xt")[:, :w]
                nc.sync.dma_start(out=xt, in_=embT[:, n0:n0 + w])
                ph1 = ps1.tile([P, ST_W], f32, tag="ph1", name="ph1")[:, :w]
                nc.tensor.matmul(ph1, lhsT=w1t[:], rhs=xt, start=True, stop=True)
                # elu(x) = max(exp(min(x,0)) - 1, x); exp(min(x,0)) = exp(-relu(-x))
                u_t = p1.tile([P, ST_W], bf16, tag="u", name="u")[:, :w]
                nc.scalar.activation(u_t, ph1, Act.Relu, scale=-1.0)
                e_t = p1.tile([P, ST_W], bf16, tag="e", name="e")[:, :w]
                nc.scalar.activation(e_t, u_t, Act.Exp, scale=-1.0)
                h1_t = p1.tile([P, ST_W], bf16, tag="h1", name="h1")[:, :w]
                nc.vector.scalar_tensor_tensor(
                    h1_t, in0=e_t, scalar=-1.0, in1=ph1,
                    op0=Alu.add, op1=Alu.max)
                pg = ps2.tile([P, ST_W], f32, tag="pg", name="pg")[:, :w]
                for b in range(nb):
                    nc.tensor.matmul(pg[:, b * H:(b + 1) * H],
                                     lhsT=h1_t[:, b * H:(b + 1) * H],
                                     rhs=w2t[:], start=True, stop=True)
                nc.scalar.activation(table[:, n0:n0 + w], pg, Act.Copy)
                sq_t = p1.tile([P, ST_W], bf16, tag="sq", name="sq")[:, :w]
                blk0 = n0 // H
                for b in range(nb):
                    nc.vector.scalar_tensor_tensor(
                        sq_t[:, b * H:(b + 1) * H],
                        in0=table[:, n0 + b * H:n0 + (b + 1) * H], scalar=0.0,
                        in1=table[:, n0 + b * H:n0 + (b + 1) * H],
                        op0=Alu.add, op1=Alu.mult,
                        accum_out=ss_all[:, blk0 + b:blk0 + b + 1])
                n0 += w
            # normalization factors + in-place normalize for this quarter
            blo, bhi = qb[qi] // H, qhi // H
            nc.scalar.activation(s_all[:, blo:bhi], ss_all[:, blo:bhi], Act.Sqrt)
            nc.vector.tensor_scalar_max(m_all[:, blo:bhi], s_all[:, blo:bhi], 1e-8)
            nc.vector.reciprocal(r_all[:, blo:bhi], m_all[:, blo:bhi])
            for blk in range(blo, bhi):
                nc.vector.tensor_scalar_mul(
                    table[:, blk * H:(blk + 1) * H],
                    table[:, blk * H:(blk + 1) * H],
                    r_all[:, blk:blk + 1])

        for qi in range(nq):
            phase1_quarter(qi)

        # --- phase 2: per-quarter row gathers + col expansion + dots ---
        chunk_id = 0
        pout = None
        for qi in range(nq):
            qcols = qb[qi + 1] * 1 - qb[qi]          # nodes in quarter
            tslice = table[:, qb[qi]:qb[qi + 1]]     # table slice (gather src)
            t0 = qstart[qi]
            while t0 < qend[qi]:
                tsz = min(GT, qend[qi] - t0)
                f2t = p2.tile([P, GT], bf16, tag="f2", name="f2")
                f2g = f2t[:, :tsz].rearrange("p (a t) -> p a t", a=1)
                nc.gpsimd.dma_gather(
                    f2g, tslice, ridx[:, t0 // 16:(t0 + tsz) // 16], tsz, tsz,
                    H, transpose=True, sbuf_tokens_per_rank=P,
                    sbuf_free_dim_per_rank=256, single_packet=False,
                    queue_num=0)
                oht = poh.tile([P, GT], bf16, tag="oh", name="oh")
                nc.sync.dma_start(out=oht[:, :tsz], in_=oh_d[:, t0:t0 + tsz])
                # pass 1: expand (PE) + multiply (DVE), software-pipelined
                prods = []
                for c0 in range(0, tsz, CHUNK):
                    c = (t0 + c0) // CHUNK
                    psF1 = psf.tile([P, CHUNK], f32, tag="psF1", name="psF1")
                    for (b, lo, hi) in chunk_segs[c]:
                        nc.tensor.matmul(
                            psF1[:, lo:hi],
                            lhsT=table[:, b * H:(b + 1) * H],
                            rhs=oht[:, c0 + lo:c0 + hi],
                            start=True, stop=True)
                    prod = pprod.tile([P, CHUNK], bf16, tag="prod")
                    nc.vector.tensor_tensor(
                        out=prod[:], in0=psF1[:],
                        in1=f2t[:, c0:c0 + CHUNK], op=Alu.mult)
                    prods.append(prod)
                # pass 2: feature-contraction reduce (PE), in chunk order
                for prod in prods:
                    g, p = divmod(chunk_id, P)
                    if p == 0:
                        pout = pso.tile([P, CHUNK], f32, tag="pout")
                    last = chunk_id == n_chunks - 1
                    nc.tensor.matmul(pout[:],
                                     lhsT=onehot[:, P - 1 - p:2 * P - 1 - p],
                                     rhs=prod[:], start=(p == 0),
                                     stop=(p == P - 1 or last))
                    chunk_id += 1
                    if p == P - 1 or last:
                        rows = p + 1
                        ost = post.tile([P, CHUNK], f32, tag="ost",
                                        name="ost")[:rows]
                        nc.vector.tensor_copy(out=ost, in_=pout[:rows])
                        nc.sync.dma_start(out=out_d[g, :rows], in_=ost)
                t0 += tsz

    nc.compile()
    return nc


def _wrap_idx(idx):
    """[S*16] int16 -> [128, S] wrapped layout (16 partitions, replicated 8x)."""
    w = idx.reshape(-1, 16).T.astype(np.int16)
    return np.tile(w, (8, 1))


def _ensure_ntff_hook():
    """Provide antenv.axon_hooks if the image lacks it (trace support only)."""
    import sys
    import types
    try:
        import antenv.axon_hooks  # noqa: F401
        return
    except ImportError:
        pass
    try:
        import antenv
        from trn_agent_boot.trn_boot import _ntff_profile_via_ctypes
        mod = types.ModuleType("antenv.axon_hooks")
        mod._hook = _ntff_profile_via_ctypes("/opt/axon/libaxon_pjrt.so")
        mod.get_axon_ntff_profile_hook = lambda: mod._hook
        mod.set_axon_ntff_profile_hook = lambda h: setattr(mod, "_hook", h)
        sys.modules["antenv.axon_hooks"] = mod
        antenv.axon_hooks = mod
    except Exception:
        pass


def kernel(emb, edge_index, W1, b1, W2, b2):
    global LAST_RESULTS
    from concourse.bass_utils import run_bass_kernel_spmd
    _ensure_ntff_hook()

    emb = np.asarray(emb, dtype=np.float32)
    W1 = np.asarray(W1, dtype=np.float32)
    W2 = np.asarray(W2, dtype=np.float32)
    b1 = np.asarray(b1, dtype=np.float32)
    b2 = np.asarray(b2, dtype=np.float32)
    assert np.abs(b1).max() == 0 and np.abs(b2).max() == 0, \
        "nonzero biases not implemented"
    col = np.asarray(edge_index[0]).astype(np.int64)
    row = np.asarray(edge_index[1]).astype(np.int64)

    n, h = emb.shape
    assert h == H
    E = col.shape[0]
    n_pad = ((n + P - 1) // P) * P
    qb = _quarter_bounds(n_pad)
    nq = len(qb) - 1
    n_blocks = n_pad // H

    # ---- host prep: (quarter, col-block) groups, core-balanced ----
    qbar = np.asarray(qb, dtype=np.int64)
    q_of = np.searchsorted(qbar[1:-1], row, side="right")
    b_of = col // H
    gkey = q_of * n_blocks + b_of
    order = np.argsort(gkey, kind="stable")       # stream order of edges
    cnt = np.bincount(gkey, minlength=nq * n_blocks)
    s_g = -(-cnt // NCORES)                       # ceil: per-core group size

    # group layout in the padded stream (q outer, b inner); pad each quarter
    # substream to a CHUNK multiple by extending its last non-empty group
    groups = []          # (q, b, start, size)
    qstart = [0] * nq
    qend = [0] * nq
    pos = 0
    for qi in range(nq):
        qstart[qi] = pos
        lastg = None
        for b in range(n_blocks):
            g = qi * n_blocks + b
            if s_g[g] == 0:
                continue
            groups.append([qi, b, pos, int(s_g[g])])
            pos += int(s_g[g])
            lastg = groups[-1]
        tail = (-pos) % CHUNK
        if tail:
            if lastg is None:
                groups.append([qi, 0, pos, tail])
            else:
                lastg[3] += tail
            pos += tail
        qend[qi] = pos
    s_pad = pos
    n_chunks = s_pad // CHUNK
    n_groups_out = math.ceil(n_chunks / P)
    groups_t = tuple((g[0], g[1], g[2], g[3]) for g in groups)
    layout = (groups_t, tuple(qstart), tuple(qend))

    key = (n_pad, layout)
    if key not in _PROG_CACHE:
        _PROG_CACHE.clear()
        _PROG_CACHE[key] = _build_program(n_pad, layout)
    nc = _PROG_CACHE[key]

    # ---- per-core streams ----
    # group base positions indexed by gkey
    gbase = np.full(nq * n_blocks, -1, dtype=np.int64)
    for (qi, b, start, size) in groups:
        g = qi * n_blocks + b
        if gbase[g] < 0:
            gbase[g] = start
    # edges of group g, in 'order', are split round-robin across cores.
    # occupancy per (g, core): cnt//8 + (core < cnt%8)
    sorted_g = gkey[order]
    # rank of each edge within its group
    grp_changes = np.flatnonzero(np.diff(sorted_g, prepend=-1))
    grp_start_in_order = np.zeros_like(sorted_g)
    grp_start_in_order[grp_changes] = np.arange(len(order))[grp_changes]
    np.maximum.accumulate(grp_start_in_order, out=grp_start_in_order)
    rank = np.arange(len(order)) - grp_start_in_order
    core_of = rank % NCORES
    slot = rank // NCORES                      # position within core's group
    stream_pos = gbase[sorted_g] + slot        # per-core stream position

    embT = np.zeros((P, n_pad), dtype=BF16)
    embT[:, :n] = emb.T.astype(BF16)
    w1t = W1.T.astype(BF16)
    w2t = W2.T.astype(BF16)

    in_maps = []
    core_edge_ids = []
    core_positions = []
    for ci in range(NCORES):
        sel = core_of == ci
        eids = order[sel]                       # original edge ids
        pospc = stream_pos[sel]
        core_edge_ids.append(eids)
        core_positions.append(pospc)
        ohm = np.zeros((P, s_pad), dtype=BF16)
        ohm[col[eids] % H, pospc] = BF16(1.0)
        ridx_flat = np.zeros(s_pad, dtype=np.int64)
        ridx_flat[pospc] = row[eids] - qbar[q_of[eids]]
        in_maps.append({
            "embT": embT, "w1t": w1t, "w2t": w2t,
            "oh": ohm, "ridx": _wrap_idx(ridx_flat),
        })

    res = run_bass_kernel_spmd(nc, in_maps, core_ids=list(range(NCORES)))
    LAST_RESULTS = res

    # ---- reassemble ----
    out = np.empty(E, dtype=np.float32)
    for ci in range(NCORES):
        stream = res.results[ci]["out"].reshape(-1)   # chunk-major dots
        out[core_edge_ids[ci]] = stream[core_positions[ci]]
    return out
